# revision 33
# baseline (speedup 1.0000x reference)
# Trainium2 Bass kernel for nn_FFF_v2 (fast-feedforward / MoE tree routing).
#
#   lam   = x @ W.T                      [B, 12] router logits
#   branch= lam > 0                      tree descent decisions
#   node  = (2^i - 1) + sum_{j<i} branch_j 2^(i-1-j)
#   out   = sum_i lam_i * Y[node_i]      [B, 4096]
#
# Sharding: data-parallel on batch across 8 cores (1024 rows each); W and Y
# replicated.  Per core:
#   - router logits via PE matmul (fp32r) on host-pre-transposed x
#   - tree-node ids via small exact fp32 matmuls (powers-of-two weights)
#   - levels 0..K_MM-1: scaled-one-hot matmul against SBUF-resident shallow
#     Y rows (PE, fp32r) -- exploits the massive node reuse at shallow levels
#   - levels K_MM..11: dma_gather of Y rows from HBM + per-partition-scalar
#     FMA (scalar_tensor_tensor) on the vector engine
import numpy as np

DEPTH = 12
B = 8192
D = 4096
N_NODES = 4095
NCORES = 8
B_LOC = B // NCORES          # 1024 rows per core

MACRO = 256                  # batch rows per macro tile
SUB = 128                    # rows per subtile (one partition block)
NSUB = MACRO // SUB          # 2
NMACRO = B_LOC // MACRO      # 4

K_MM = 8                     # levels 0..K_MM-1 handled by one-hot matmul
N_SH = 2 ** K_MM - 1         # shallow nodes (255)
NCHUNK = (N_SH + 127) // 128  # 2
N_GL = DEPTH - K_MM          # gather levels (4)

# Router: x and W are shipped as fp16 (hi, lo) split pairs, packed so ONE
# fp16 matmul per 128-feature chunk computes all four hi/lo cross terms:
#   lhsT = [W_hi | W_lo]  [128, 24]
#   rhs  = [x_hi | x_lo]  [128, 2*MACRO]
#   lam  = sum of the four [12, MACRO] blocks of the [24, 2*MACRO] PSUM
# Exact to fp32-accumulation precision, at 1 cycle/row (4x faster than fp32).
Y_BF16 = True                 # Y (gather + shallow) in bf16: halves gather DMA
OUT_BF16 = True               # accumulate + store out in bf16, upcast on host
DEBUG_IDX = False            # emit idx16/idxr debug outputs

_CACHE = {}


def _level_of(n):
    # level i spans nodes [2^i - 1, 2^(i+1) - 1)
    lev = 0
    while n >= 2 ** (lev + 1) - 1:
        lev += 1
    return lev


def _host_consts():
    # powT[j, i] = powmat[i, j] = 2^(i-1-j) for j < i  (lhsT of prefix matmul)
    powT = np.zeros((DEPTH, DEPTH), np.float32)
    for i in range(DEPTH):
        for j in range(i):
            powT[j, i] = float(1 << (i - 1 - j))
    # offs_w[p, l*8+f] = 2^l - 1 (level offset in wrapped (level, slot) layout)
    offs = np.array([(1 << i) - 1 for i in range(DEPTH)], np.float32)
    offs_w = np.broadcast_to(
        np.repeat(offs, SUB // 16)[None, :], (16, DEPTH * (SUB // 16))
    ).copy()
    # bselT[l, c*128+p] = 1 if level(c*128+p) == l else 0   (lhsT of bc matmul)
    bselT = np.zeros((DEPTH, NCHUNK * 128), np.float32)
    # nrel[p, c] = node - (2^level - 1), or -1 for pad positions
    nrel = np.full((128, NCHUNK), -1.0, np.float32)
    for c in range(NCHUNK):
        for p in range(128):
            n = c * 128 + p
            if n < N_SH:
                lev = _level_of(n)
                bselT[lev, c * 128 + p] = 1.0
                nrel[p, c] = float(n - ((1 << lev) - 1))
    # identity for PE transpose of the stacked [24, MACRO] lam/prefix tile
    ident = np.eye(2 * DEPTH, dtype=np.float32)
    # diag mask for per-row scaling matmuls (deep-level accumulation on PE)
    dmask = np.eye(128, dtype=np.float32)
    return powT, offs_w, bselT, nrel, ident, dmask


def _build_program():
    import concourse.bass as bass
    import concourse.bacc as bacc
    import concourse.mybir as mybir
    import concourse.tile as tile
    from contextlib import ExitStack

    dt = mybir.dt
    f32 = dt.float32
    f32r = dt.float32r
    bf16 = dt.bfloat16
    f16 = dt.float16
    i16 = dt.int16
    Alu = mybir.AluOpType

    nc = bacc.Bacc(trn_type="TRN2", num_swdge_queues=4)

    ydt = bf16 if Y_BF16 else f32
    odt = bf16 if OUT_BF16 else f32
    xt_d = nc.dram_tensor("xt", [NMACRO, 128, 32, 2, MACRO], f16, kind="ExternalInput")
    y_d = nc.dram_tensor("y", [N_NODES, D], ydt, kind="ExternalInput")
    # 44 stationary cols: W_hi at 0..11, W_lo at 32..43 (zeros between) so the
    # two PSUM row blocks start at quadrant-aligned partitions 0 and 32.
    WSTK = 32 + DEPTH
    wt_d = nc.dram_tensor("wt", [128, 32, WSTK], f16, kind="ExternalInput")
    powt_d = nc.dram_tensor("powt", [DEPTH, DEPTH], f32, kind="ExternalInput")
    offsw_d = nc.dram_tensor(
        "offsw", [16, DEPTH * (SUB // 16)], f32, kind="ExternalInput"
    )
    bselt_d = nc.dram_tensor("bselt", [DEPTH, NCHUNK * 128], f32, kind="ExternalInput")
    nrel_d = nc.dram_tensor("nrel", [128, NCHUNK], f32, kind="ExternalInput")
    ident_d = nc.dram_tensor("ident", [2 * DEPTH, 2 * DEPTH], f32, kind="ExternalInput")
    dmask_d = nc.dram_tensor("dmask", [128, 128], ydt, kind="ExternalInput")
    out_d = nc.dram_tensor("out", [B_LOC, D], odt, kind="ExternalOutput")
    if DEBUG_IDX:
        dbg16_d = nc.dram_tensor(
            "dbg16", [NMACRO * NSUB, 16, DEPTH * (SUB // 16)], i16,
            kind="ExternalOutput",
        )
        dbgr_d = nc.dram_tensor(
            "dbgr", [NMACRO * NSUB, 128, N_GL * (SUB // 16)], i16,
            kind="ExternalOutput",
        )

    with tile.TileContext(nc) as tc, ExitStack() as ctx:
        consts = ctx.enter_context(tc.tile_pool(name="consts", bufs=1))
        xt_p = ctx.enter_context(tc.tile_pool(name="xt", bufs=2))
        small = ctx.enter_context(tc.tile_pool(name="small", bufs=4))
        small4 = ctx.enter_context(tc.tile_pool(name="small4", bufs=6))
        st_p = ctx.enter_context(tc.tile_pool(name="st", bufs=NMACRO * NCHUNK))
        g_p = ctx.enter_context(tc.tile_pool(name="g", bufs=6))
        diag_p = ctx.enter_context(tc.tile_pool(name="diag", bufs=8))
        out_p = ctx.enter_context(tc.tile_pool(name="outp", bufs=2))
        dram_p = ctx.enter_context(tc.tile_pool(name="idxd", bufs=8, space="DRAM"))
        ps_lam = ctx.enter_context(tc.tile_pool(name="pslam", bufs=2, space="PSUM"))
        ps_bc = ctx.enter_context(tc.tile_pool(name="psbc", bufs=2, space="PSUM"))
        ps_tp = ctx.enter_context(tc.tile_pool(name="pstp", bufs=1, space="PSUM"))
        ps_out = ctx.enter_context(tc.tile_pool(name="psout", bufs=3, space="PSUM"))

        # ---- critical-path constant: router weights only ----
        wt_sb = consts.tile([128, 32, WSTK], f16)
        nc.sync.dma_start(wt_sb[:], wt_d.ap())

        # ================= pass A: routers + tree state for ALL macros ======
        # One continuous PE stream (keeps the tensor engine p-state warm);
        # pb/bc of macro m are queued after router m+1 so the branch-bit
        # round trip through DVE hides under the next router.
        lamTs, branches, pfxTs, sts = [], [], [], []
        consts_sb = {}

        def emit_pb_bc(m):
            # prefix^T [12, MACRO] = powmat @ branch  (exact fp32)
            pb_ps = ps_bc.tile([DEPTH, MACRO], f32, tag="bc", name="pb_ps")
            nc.tensor.matmul(
                pb_ps[:], consts_sb["powt"][:], branches[m][:], start=True, stop=True
            )
            pfxT = small.tile([DEPTH, MACRO], f32, tag="pfxT", name="pfxT")
            nc.scalar.copy(pfxT[:], pb_ps[:])
            pfxTs.append(pfxT)

            # ---- S^T build: one chunk of 128 shallow nodes at a time ----
            st = []
            bselt_sb = consts_sb["bselt"]
            nrel_sb = consts_sb["nrel"]
            for c in range(NCHUNK):
                bc_ps = ps_bc.tile([128, 2 * MACRO], f32, tag="bc", name="bc_ps")
                nc.tensor.matmul(
                    bc_ps[:, :MACRO], bselt_sb[:, c * 128 : (c + 1) * 128],
                    pfxT[:], start=True, stop=True,
                )
                nc.tensor.matmul(
                    bc_ps[:, MACRO:], bselt_sb[:, c * 128 : (c + 1) * 128],
                    lamTs[m][:], start=True, stop=True,
                )
                lbc = small.tile([128, MACRO], f32, tag="lbc", name="lbc")
                nc.scalar.copy(lbc[:], bc_ps[:, MACRO:])
                stc = st_p.tile([128, MACRO], ydt, tag="st", name="stc")
                nc.vector.scalar_tensor_tensor(
                    stc[:], bc_ps[:, :MACRO], nrel_sb[:, c : c + 1], lbc[:],
                    Alu.is_equal, Alu.mult,
                )
                st.append(stc)
            sts.append(st)

        for m in range(NMACRO):
            # ---- load x^T macro tile [128, 32, 2, MACRO] (fp16 hi|lo) ----
            xt = xt_p.tile([128, 32, 2, MACRO], f16, tag="xt")
            for h in range(4):
                nc.sync.dma_start(
                    xt[:, 8 * h : 8 * (h + 1)], xt_d.ap()[m][:, 8 * h : 8 * (h + 1)]
                )

            # ---- router: [44, 2*MACRO] PSUM = [Whi|0|Wlo] @ [xhi|xlo] ----
            lam_ps = ps_lam.tile([WSTK, 2 * MACRO], f32, tag="lam")
            for c in range(32):
                nc.tensor.matmul(
                    lam_ps[:],
                    wt_sb[:, c, :],
                    xt[:, c].rearrange("p h j -> p (h j)"),
                    start=(c == 0), stop=(c == 31),
                )
            # lam = sum of the four [12, MACRO] blocks (exact fp32).
            # TT may read only one PSUM input: stage the hi block via scalar.
            lam_c = small.tile([DEPTH, 2 * MACRO], f32, tag="lamc", bufs=2)
            nc.scalar.copy(lam_c[:], lam_ps[:DEPTH, :])
            lam_r = small.tile([DEPTH, 2 * MACRO], f32, tag="lamr", bufs=2)
            nc.vector.tensor_tensor(
                lam_r[:], lam_c[:], lam_ps[32 : 32 + DEPTH, :], Alu.add
            )

            if m == 0:
                # remaining constants, deferred so the first x tile and the
                # router aren't stuck behind ~4.5 MiB of const DMA traffic
                powt_sb = consts.tile([DEPTH, DEPTH], f32)
                nc.sync.dma_start(powt_sb[:], powt_d.ap())
                offsw_sb = consts.tile([16, DEPTH * (SUB // 16)], f32)
                nc.sync.dma_start(offsw_sb[:], offsw_d.ap())
                bselt_sb = consts.tile([DEPTH, NCHUNK * 128], f32)
                nc.sync.dma_start(bselt_sb[:], bselt_d.ap())
                nrel_sb = consts.tile([128, NCHUNK], f32)
                nc.sync.dma_start(nrel_sb[:], nrel_d.ap())
                ident_sb = consts.tile([2 * DEPTH, 2 * DEPTH], f32)
                nc.sync.dma_start(ident_sb[:], ident_d.ap())
                dmask_sb = consts.tile([128, 128], ydt)
                nc.sync.dma_start(dmask_sb[:], dmask_d.ap())
                ysh_sb = consts.tile([128, NCHUNK * D], ydt)
                for c in range(NCHUNK):
                    lo = c * 128
                    hi = min(lo + 128, N_NODES)
                    nc.scalar.dma_start(
                        ysh_sb[: hi - lo, c * D : (c + 1) * D], y_d.ap()[lo:hi, :]
                    )
                consts_sb.update(
                    powt=powt_sb, bselt=bselt_sb, nrel=nrel_sb
                )

            # branch bits and lam^T in SBUF (partition 0 based)
            lamT = small.tile([DEPTH, MACRO], f32, tag="lamT")
            nc.vector.tensor_tensor(
                lamT[:], lam_r[:, :MACRO], lam_r[:, MACRO:], Alu.add
            )
            lamTs.append(lamT)
            branch = small.tile([DEPTH, MACRO], f32, tag="branch", bufs=2)
            nc.vector.tensor_scalar(branch[:], lamT[:], 0.0, None, Alu.is_gt)
            branches.append(branch)

            if m >= 1:
                emit_pb_bc(m - 1)
        emit_pb_bc(NMACRO - 1)

        # ================= pass B: gather + accumulate per subtile ==========
        for m in range(NMACRO):
            lamT = lamTs[m]
            pfxT = pfxTs[m]
            st = sts[m]
            for s in range(NSUB):
                bsl = slice(s * SUB, (s + 1) * SUB)
                # ---- lam to batch-partition layout ----
                # (plain identity matmul: out = in.T @ I; avoids the PE
                # transpose mode, which corrupts partitions after fp32r MMs)
                tpw = ps_tp.tile([SUB, 128], f32, tag="tpw")
                tp_ps = tpw[:, :DEPTH]
                nc.tensor.matmul(
                    tp_ps, lamT[:, bsl], ident_sb[:DEPTH, :DEPTH],
                    start=True, stop=True,
                )
                lamb = small4.tile([SUB, DEPTH], f32, tag="lamb")
                nc.vector.tensor_copy(lamb[:], tp_ps)

                # ---- node ids in the 16-partition-wrapped (level, slot)
                # layout dma_gather wants, via per-16-column PE transposes ----
                w_ps = tpw[:16, 16 : 16 + (SUB // 16) * DEPTH].rearrange(
                    "p (f l) -> p f l", f=SUB // 16
                )
                for f in range(SUB // 16):
                    nc.tensor.matmul(
                        w_ps[:, f, :],
                        pfxT[:, s * SUB + f * 16 : s * SUB + (f + 1) * 16],
                        ident_sb[:DEPTH, :DEPTH],
                        start=True, stop=True,
                    )
                idx16 = small4.tile([16, DEPTH, SUB // 16], i16, tag="idx16")
                nc.vector.tensor_tensor(
                    idx16[:], w_ps[:].rearrange("p f l -> p l f"), offsw_sb[:],
                    Alu.add,
                )
                # replicate to all 8 Q7 descriptor-gen cores via a DRAM bounce
                idxd = dram_p.tile([16, N_GL * (SUB // 16)], i16, tag="idxd")
                nc.sync.dma_start(
                    idxd[:], idx16[:, K_MM:, :].rearrange("p l f -> p (l f)")
                )
                idxr = small4.tile([128, N_GL, SUB // 16], i16, tag="idxr")
                for gq in range(8):
                    nc.sync.dma_start(
                        idxr[16 * gq : 16 * (gq + 1), :, :].rearrange(
                            "p l f -> p (l f)"
                        ),
                        idxd[:],
                    )
                if DEBUG_IDX:
                    nc.sync.dma_start(dbg16_d.ap()[m * NSUB + s], idx16[:].rearrange("p l f -> p (l f)"))
                    nc.sync.dma_start(dbgr_d.ap()[m * NSUB + s], idxr[:].rearrange("p l f -> p (l f)"))

                # ---- gather deep levels from HBM ----
                gt = []
                for li in range(N_GL):
                    g = g_p.tile([128, 1, D], ydt, tag="g")
                    y_ap = y_d.ap() if ydt != f32r else y_d.ap().bitcast(f32)
                    nc.gpsimd.dma_gather(
                        g[:], y_ap, idxr[:, li, :], SUB, SUB, D,
                        queue_num=li % 4,
                    )
                    gt.append(g)

                # diag(lam_l) for the deep-level scaling matmuls (DVE; scalar
                # engine is loaded with the PSUM->SBUF output copies)
                diags = []
                for li in range(N_GL):
                    dg = diag_p.tile([128, 128], ydt, tag="diag")
                    nc.vector.tensor_scalar(
                        dg[:], dmask_sb[:], lamb[:, K_MM + li : K_MM + li + 1],
                        None, Alu.mult,
                    )
                    diags.append(dg)

                out_t = out_p.tile([SUB, D], odt, tag="out")
                for q in range(D // 512):
                    qsl = slice(q * 512, (q + 1) * 512)
                    # one accumulation group in PSUM: shallow one-hot matmul
                    # + per-row-scaled gathered rows (diag matmuls)
                    po = ps_out.tile([SUB, 512], f32, tag="po")
                    for c in range(NCHUNK):
                        nc.tensor.matmul(
                            po[:], st[c][:, bsl],
                            ysh_sb[:, c * D + q * 512 : c * D + (q + 1) * 512],
                            start=(c == 0), stop=False,
                        )
                    for li in range(N_GL):
                        nc.tensor.matmul(
                            po[:], diags[li][:], gt[li][:, 0, qsl],
                            start=False, stop=(li == N_GL - 1),
                        )
                    if q % 2 == 0:
                        nc.scalar.copy(out_t[:, qsl], po[:])
                    else:
                        nc.vector.tensor_copy(out_t[:, qsl], po[:])
                nc.scalar.dma_start(out_d.ap()[m * MACRO + s * SUB :][:SUB, :], out_t[:])

    nc.compile()
    return nc


def _patch_walrus_passes():
    # The default walrus pass list in this environment omits
    # lower_custom_kernel, which the Pool custom instructions (dma_gather)
    # need. Inject it in front of codegen.
    import concourse.bass_utils as bu

    if getattr(bu, "_ant_lck_patched", False):
        return
    bu._ant_lck_patched = True
    orig = bu.run_command

    def run_command(argv, **kw):
        if argv and "walrus_driver" in str(argv[0]):
            argv = list(argv)
            for i, a in enumerate(argv):
                if a == "--pass" and "lower_custom_kernel" not in argv[i + 1]:
                    argv[i + 1] = argv[i + 1].replace(
                        "codegen", "lower_custom_kernel,codegen"
                    )
                    break
        return orig(argv, **kw)

    bu.run_command = run_command


def _get_program():
    if "nc" not in _CACHE:
        _CACHE["nc"] = _build_program()
    return _CACHE["nc"]


def _prep_in_maps(x, W, Y):
    powT, offs_w, bselT, nrel, ident, dmask = _host_consts()
    if Y_BF16:
        import ml_dtypes

        Y = np.ascontiguousarray(Y).astype(ml_dtypes.bfloat16)
        dmask = dmask.astype(ml_dtypes.bfloat16)
    else:
        Y = np.ascontiguousarray(Y, np.float32)
    # W as stacked fp16 (hi | 0 | lo) planes: wt[p, c, 0:12]=W_hi, [32:44]=W_lo
    W = np.asarray(W, np.float32)
    w_hi = W.astype(np.float16)
    w_lo = (W - w_hi.astype(np.float32)).astype(np.float16)
    ws = np.zeros((32 + DEPTH, D), np.float16)
    ws[:DEPTH] = w_hi
    ws[32:] = w_lo
    wt = np.ascontiguousarray(ws.T.reshape(32, 128, 32 + DEPTH).transpose(1, 0, 2))
    x = np.asarray(x, np.float32)
    x_hi = x.astype(np.float16)
    x_lo = (x - x_hi.astype(np.float32)).astype(np.float16)
    in_maps = []
    xhr = x_hi.reshape(NCORES, B_LOC, D)
    xlr = x_lo.reshape(NCORES, B_LOC, D)
    for c in range(NCORES):
        xs = np.stack([xhr[c], xlr[c]], axis=0)  # [2, B_LOC, D]
        # target [m, p, cchunk, h, j]: elem = xs[h, m*MACRO+j, cchunk*128+p]
        a = xs.reshape(2, NMACRO, MACRO, 32, 128)
        xtm = np.ascontiguousarray(a.transpose(1, 4, 3, 0, 2))
        in_maps.append(
            {
                "xt": xtm, "y": Y, "wt": wt, "powt": powT, "offsw": offs_w,
                "bselt": bselT, "nrel": nrel, "ident": ident, "dmask": dmask,
            }
        )
    return in_maps


def kernel(x, W, Y, _trace=False):
    from concourse.bass_utils import run_bass_kernel_spmd

    _patch_walrus_passes()

    nc = _get_program()
    in_maps = _prep_in_maps(np.asarray(x), np.asarray(W), np.asarray(Y))
    res = run_bass_kernel_spmd(nc, in_maps, list(range(NCORES)), trace=_trace)
    out = np.concatenate(
        [np.asarray(res.results[c]["out"]).astype(np.float32) for c in range(NCORES)],
        axis=0,
    )
    if _trace:
        _CACHE["last_result"] = res
    return out

